# revision 6
# baseline (speedup 1.0000x reference)
"""Trainium2 Bass kernel for GAP -> tiny Mamba (channel attention) -> broadcast multiply.

Reference computation (per batch):
    pooled = mean(x1 over H,W)                  # [C] ; sequence of length C=512, d_model=1
    att    = mamba(pooled)                      # d_inner=2, d_state=16, dt_rank=1, conv=4
    out    = x2 * att[None, None, :]

Sharding: data-parallel over batch B=16 across 8 cores (2 batches/core), params
replicated. Memory-bound: 48 MiB/core of HBM traffic (x1+x2 reads, out writes)
at ~432 GB/s aggregate (reads and writes share one pool) -> ~117 us DMA floor.

v4 design. v3 (172 us) showed Vector as co-bottleneck: GAP accumulate+fold
(54 us) + phase-2 (44 us) + chains (~16 us) ~= 114 us ~= the DMA floor, and
the serialization pushed attP(1) to 139 us leaving a 22 us unoverlapped tail.
Fixes:
  - GAP accumulation moved into the DMA engines: x1 tile 0 lands in the
    [128,4096] accumulator via a sync-ring bypass DMA; tiles 1-3 accumulate
    into it with gpsimd software-DGE transfers (accum_op=add). The framework's
    WAW tracking serializes them (race-free); Vector keeps only the 4-op
    column fold per batch (~5.5 us).
  - No x1 tile pool at all; x2 gets 8 dedicated buffers. NO buffer reuse
    anywhere in phase 1 -> no ring-head blocking; every read streams as soon
    as the queues are free.
  - Vector total ~60 us << floor: folds + chain smalls + 16 half-tile phase-2
    multiplies; each half is written out right after its multiply.
  - Setup stays off Vector (sign of A folded into the negated dt-selector
    half of bsel64); weight DMA descgen split across GpSimd/Scalar queues.
PSUM slot rotation, Taylor softplus, tensor_tensor_scan, quadrant-aligned
small matmuls unchanged from v2.
"""

import os
import numpy as np

import concourse.bass as bass
import concourse.bacc as bacc
import concourse.tile as tile
from concourse import mybir
from concourse.bass_utils import run_bass_kernel_spmd

F32 = mybir.dt.float32
AF = mybir.ActivationFunctionType
OP = mybir.AluOpType

N_CORES = 8
B_FULL, H, W, C = 16, 64, 64, 512
B_LOC = B_FULL // N_CORES            # 2 batches per core
HW = H * W                           # 4096 spatial positions
Q = 8                                # image rows per partition per stream tile
ROWS_PER_TILE = 128 * Q              # 1024
N_TILES = HW // ROWS_PER_TILE        # 4 tiles per batch image
CK = 1024                            # vector chunk width (columns)

LN2 = 0.6931471805599453

WEIGHT_SHAPES = {
    "in_proj_w": [4, 1],
    "conv_w": [2, 1, 4],
    "conv_b": [2],
    "x_proj_w": [33, 2],
    "dt_proj_w": [2, 1],
    "dt_proj_b": [2],
    "A_log": [2, 16],
    "Dp": [2],
    "out_proj_w": [1, 2],
}

LAST_RESULTS = None
_CACHE = {}


def _dap(handle, offset, pattern):
    return bass.AP(handle, offset, pattern)


def _build():
    nc = bacc.Bacc(None, target_bir_lowering=False, dynamic_dma_scratch_size=32768)

    x1h = nc.dram_tensor("x1", [B_LOC, H, W, C], F32, kind="ExternalInput")
    x2h = nc.dram_tensor("x2", [B_LOC, H, W, C], F32, kind="ExternalInput")
    wh = {
        name: nc.dram_tensor(name, shape, F32, kind="ExternalInput")
        for name, shape in WEIGHT_SHAPES.items()
    }
    outh = nc.dram_tensor("out", [B_LOC, H, W, C], F32, kind="ExternalOutput")

    # ---- inline 0/1 constants ----
    # [2,64] broadcast selector: cols 0:32 row d -> out rows (d,s) with -1 (for
    # dt; bakes in the sign of A = -exp(A_log)), cols 32:64 with +1 (for g).
    bsel_np = np.zeros((2, 64), np.float32)
    for d in range(2):
        bsel_np[d, 16 * d : 16 * d + 16] = -1.0
        bsel_np[d, 32 + 16 * d : 48 + 16 * d] = 1.0
    bsel_d = nc.inline_tensor(bsel_np, "c_bsel64")
    # [32,2] reduce-s selector: row (d,s) -> col d
    rsel_np = np.zeros((32, 2), np.float32)
    for d in range(2):
        rsel_np[16 * d : 16 * d + 16, d] = 1.0
    rsel_d = nc.inline_tensor(rsel_np, "c_rsel32")

    def img_ap(handle, b, t, half):
        # [128, Q*C/2] view of half `half` of image rows [t*1024, (t+1)*1024).
        off = (b * HW + t * ROWS_PER_TILE) * C + half * (Q * C // 2)
        return _dap(handle, off, [[Q * C, 128], [1, Q * C // 2]])

    with tile.TileContext(nc) as tc:
        with (
            tc.tile_pool(name="work", bufs=1) as work,
            tc.tile_pool(name="x2pool", bufs=8) as x2pool,
            tc.tile_pool(name="psum", bufs=6, space="PSUM") as psum,
            tc.tile_pool(name="psum_att", bufs=2, space="PSUM") as psum_att,
        ):
            # ========== setup: constants & weight-derived tiles ==========
            # Vector gets NONE of this: its program must start with the GAP
            # adds so the x1 pipeline is never gated on weight prep.
            bsel64 = work.tile([2, 64], F32)
            nc.gpsimd.dma_start(out=bsel64[:], in_=bsel_d.ap())
            rsel32 = work.tile([32, 2], F32)
            nc.gpsimd.dma_start(out=rsel32[:], in_=rsel_d.ap())

            # a32 = +exp(A_log) on rows (d,s); sign lives in bsel64 cols 0:32
            a32 = work.tile([32, 1], F32)
            nc.scalar.dma_start(out=a32[:], in_=_dap(wh["A_log"], 0, [[1, 32], [1, 1]]))
            nc.scalar.activation(a32[:], a32[:], AF.Exp)

            cb2 = work.tile([2, 1], F32)       # conv_b
            nc.scalar.dma_start(out=cb2[:], in_=_dap(wh["conv_b"], 0, [[1, 2], [1, 1]]))
            dp2 = work.tile([2, 1], F32)       # Dp
            nc.scalar.dma_start(out=dp2[:], in_=_dap(wh["Dp"], 0, [[1, 2], [1, 1]]))

            # conv taps 0..2 (raw: the xr rows already carry the in_proj weight)
            wq = work.tile([2, 4], F32)
            nc.gpsimd.dma_start(out=wq[:], in_=_dap(wh["conv_w"], 0, [[4, 2], [1, 4]]))

            # stat6 [128,6]: cols = [win0, win1, wz0, wz1, win0*cw03, win1*cw13]/HW
            # broadcast to all 128 partitions; used as three [128,2] stationaries
            # (GAP-reduce + in_proj for the xr rows, z rows, conv-tap3 init rows).
            w6 = work.tile([1, 6], F32)
            nc.gpsimd.dma_start(out=w6[0:1, 0:4], in_=_dap(wh["in_proj_w"], 0, [[0, 1], [1, 4]]))
            cw3 = work.tile([1, 2], F32)
            nc.gpsimd.dma_start(out=cw3[:], in_=_dap(wh["conv_w"], 3, [[0, 1], [4, 2]]))
            nc.gpsimd.tensor_mul(w6[0:1, 4:6], w6[0:1, 0:2], cw3[:])
            nc.scalar.mul(w6[:], w6[:], 1.0 / HW)
            stat6 = work.tile([128, 6], F32)
            nc.gpsimd.partition_broadcast(stat6[:], w6[:])

            # stat66 [3,66]: moving rows (xconv d0, xconv d1, ones).
            # cols 0-1:  dt_pre rows (d): xp_dt[d']*dtw[d] (+ dtb[d] via ones row)
            # cols 2-33:  B rows (d,s): xp_B[s, d']
            # cols 34-65: C rows (d,s): xp_C[s, d']
            stat66 = work.tile([3, 66], F32)
            nc.gpsimd.memset(stat66[:], 0.0)
            xpdt2 = work.tile([2, 1], F32)
            nc.scalar.dma_start(out=xpdt2[:], in_=_dap(wh["x_proj_w"], 0, [[1, 2], [1, 1]]))
            dtwbc = work.tile([2, 2], F32)
            nc.scalar.dma_start(out=dtwbc[:], in_=_dap(wh["dt_proj_w"], 0, [[0, 2], [1, 2]]))
            nc.scalar.mul(stat66[0:2, 0:2], dtwbc[:], xpdt2[:])
            nc.scalar.dma_start(out=stat66[2:3, 0:2], in_=_dap(wh["dt_proj_b"], 0, [[0, 1], [1, 2]]))
            for d in range(2):
                nc.scalar.dma_start(
                    out=stat66[0:2, 2 + 16 * d : 18 + 16 * d],
                    in_=_dap(wh["x_proj_w"], 2, [[1, 2], [2, 16]]),
                )
                nc.scalar.dma_start(
                    out=stat66[0:2, 34 + 16 * d : 50 + 16 * d],
                    in_=_dap(wh["x_proj_w"], 34, [[1, 2], [2, 16]]),
                )

            # wout_bc [2,128]: every col = out_proj_w; fuses out_proj with the
            # broadcast of att to 128 partitions.
            wout2 = work.tile([2, 1], F32)
            nc.scalar.dma_start(out=wout2[:], in_=_dap(wh["out_proj_w"], 0, [[1, 2], [1, 1]]))
            wout_bc = work.tile([2, 128], F32)
            nc.gpsimd.tensor_copy(
                wout_bc[:], bass.AP(wout2.tensor, wout2.offset, [wout2.ap[0], [0, 128]])
            )

            # xconv moving tiles [3, C]: rows 0-1 = silu(conv), row 2 = ones.
            xconv3 = []
            for b in range(2):
                xc = work.tile([3, C], F32, tag=f"xconv{b}")
                nc.gpsimd.memset(xc[:], 1.0)     # row 2 stays 1.0
                xconv3.append(xc)

            # Vector = pure data path; Scalar triggers the out writes.
            ENG = [nc.vector, nc.vector]
            TRIG = [nc.scalar, nc.scalar]

            # ========== phase 1: reads (x1 priority, then x2) ==========
            accs = []
            for b in range(2):
                acc = work.tile([128, Q * C], F32, tag=f"acc{b}")
                accs.append(acc)
            # x1 tile 0 -> acc (bypass, HWDGE); tiles 1-3 accumulate straight
            # into acc via gpsimd software-DGE (the only engine with accum_op).
            # WAW tracking serializes the per-acc transfers; wave order
            # (t1 b0, t1 b1, t2 b0, ...) keeps both accs' chains in flight.
            for b in range(2):
                nc.sync.dma_start(
                    out=accs[b][:],
                    in_=_dap(x1h, b * HW * C, [[Q * C, 128], [1, Q * C]]),
                )
            # accum transfers are split into [128,2048] column halves: the
            # software-DGE accum path wedges on per-partition runs > 8 KiB.
            for t in range(1, N_TILES):
                for b in range(2):
                    for hf in range(2):
                        nc.gpsimd.dma_start(
                            out=accs[b][:, 2048 * hf : 2048 * (hf + 1)],
                            in_=_dap(
                                x1h,
                                (b * HW + t * ROWS_PER_TILE) * C + hf * 2048,
                                [[Q * C, 128], [1, 2048]],
                            ),
                            accum_op=OP.add,
                        )
            x2tiles = {}
            for b in range(2):
                for t in range(N_TILES):
                    x2t = x2pool.tile([128, Q * C], F32, tag="x2t")
                    nc.sync.dma_start(
                        out=x2t[:],
                        in_=_dap(x2h, (b * HW + t * ROWS_PER_TILE) * C, [[Q * C, 128], [1, Q * C]]),
                    )
                    x2tiles[(b, t)] = x2t

            # GAP column fold [128,4096] -> [128,512], chunked to ~1 us ops.
            for b in range(2):
                E = ENG[b]
                aa = accs[b]
                E.tensor_add(aa[:, 0:1024], aa[:, 0:1024], aa[:, 2048:3072])
                E.tensor_add(aa[:, 1024:2048], aa[:, 1024:2048], aa[:, 3072:4096])
                E.tensor_add(aa[:, 0:1024], aa[:, 0:1024], aa[:, 1024:2048])
                E.tensor_add(aa[:, 0:512], aa[:, 0:512], aa[:, 512:1024])

            # ========== per-batch mamba chain ==========
            # After the GAP matmuls the [128,4096] accumulator is scratch; the
            # chain's [*,512] temporaries alias into its 8 column slots.
            def slot(b, k, p=32):
                return accs[b][0:p, 512 * k : 512 * (k + 1)]

            def mamba(b):
                E = ENG[b]
                xc = xconv3[b]
                aa = accs[b]
                # GAP reduce + in_proj (+ conv tap3): three [2, C] psum rows
                gapXr = psum.tile([2, C], F32, tag="pp")
                nc.tensor.matmul(gapXr[:], stat6[:, 0:2], aa[:, 0:512], start=True, stop=True)
                gapZ = psum.tile([2, C], F32, tag="pp")
                nc.tensor.matmul(gapZ[:], stat6[:, 2:4], aa[:, 0:512], start=True, stop=True)
                gapCi = psum.tile([2, C], F32, tag="pp")
                nc.tensor.matmul(gapCi[:], stat6[:, 4:6], aa[:, 0:512], start=True, stop=True)
                # causal conv: cacc = cinit; taps 2,1,0 read xr straight from PSUM
                cacc = slot(b, 5, 2)
                E.tensor_copy(cacc, gapCi[:])
                for j in (2, 1, 0):
                    s = 3 - j
                    E.scalar_tensor_tensor(
                        cacc[:, s:C], gapXr[:, 0 : C - s], wq[:, j : j + 1],
                        cacc[:, s:C], op0=OP.mult, op1=OP.add,
                    )
                # xconv = silu(conv + conv_b); sz = silu(z) straight from PSUM
                sz = slot(b, 6, 2)
                nc.scalar.activation(xc[0:2, :], cacc, AF.Silu, bias=cb2[:])
                nc.scalar.activation(sz, gapZ[:], AF.Silu)
                # x_proj + dt_proj(+bias): three base-0 psum tiles
                xdtP = psum.tile([2, C], F32, tag="pp")
                nc.tensor.matmul(xdtP[:], stat66[:, 0:2], xc[:], start=True, stop=True)
                xbP = psum.tile([32, C], F32, tag="pp")
                nc.tensor.matmul(xbP[:], stat66[:, 2:34], xc[:], start=True, stop=True)
                xcP = psum.tile([32, C], F32, tag="pp")
                nc.tensor.matmul(xcP[:], stat66[:, 34:66], xc[:], start=True, stop=True)
                bm = slot(b, 0)
                E.tensor_copy(bm, xbP[:])
                # dt = softplus(dt_pre) ~= ln2 + x/2 + x^2*(1/8 - x^2/192)
                t2a = slot(b, 3, 2)
                t2b = slot(b, 4, 2)
                t2c = slot(b, 5, 2)     # cacc is dead after the silu
                dt2 = slot(b, 7, 2)
                E.tensor_copy(t2a, xdtP[:])
                E.tensor_mul(t2b, t2a, t2a)
                E.tensor_scalar(t2c, t2b, -1.0 / 192.0, 0.125, op0=OP.mult, op1=OP.add)
                E.tensor_mul(t2c, t2c, t2b)
                E.tensor_scalar(t2a, t2a, 0.5, LN2, op0=OP.mult, op1=OP.add)
                E.tensor_add(dt2, t2c, t2a)
                g2 = slot(b, 5, 2)      # t2c is dead after dt2
                E.tensor_mul(g2, dt2, xc[0:2, :])        # g = dt*xconv
                # broadcast dt,g to (d,s) lanes; the dt selector is negated so
                # da = exp(a32 * (-dt)) = exp(A * dt) with a32 = +exp(A_log)
                dag1P = psum.tile([32, C], F32, tag="pp")
                nc.tensor.matmul(dag1P[:], bsel64[:, 0:32], dt2, start=True, stop=True)
                dag2P = psum.tile([32, C], F32, tag="pp")
                nc.tensor.matmul(dag2P[:], bsel64[:, 32:64], g2, start=True, stop=True)
                da = slot(b, 7)         # dt2 rows are dead after dag1P
                nc.scalar.activation(da, dag1P[:], AF.Exp, scale=a32[:])
                dbu = slot(b, 1)
                E.tensor_mul(dbu, dag2P[:], bm)
                # selective scan h[:,t] = dA[:,t]*h[:,t-1] + dBu[:,t]
                h = slot(b, 2)
                E.tensor_tensor_scan(h, da, dbu, 0.0, op0=OP.mult, op1=OP.add)
                hc = slot(b, 1)         # dbu dead after the scan
                E.tensor_mul(hc, h, xcP[:])
                y2P = psum.tile([2, C], F32, tag="pp")
                nc.tensor.matmul(y2P[:], rsel32[:], hc, start=True, stop=True)
                # y = (y + Dp*xconv) * silu(z); att = out_proj(y) broadcast
                yg = slot(b, 3, 2)      # t2a dead after dt2
                E.scalar_tensor_tensor(yg, xc[0:2, :], dp2[:], y2P[:], op0=OP.mult, op1=OP.add)
                E.tensor_mul(yg, yg, sz)
                # att lives in its own 2-bank pool: it stays live through all
                # of the batch's phase-2 multiplies and must not gate the other
                # batch's psum rotation.
                attP = psum_att.tile([128, C], F32, tag="att")
                nc.tensor.matmul(attP[:], wout_bc[:], yg, start=True, stop=True)
                return attP

            att_tiles = [mamba(0), mamba(1)]

            # ========== phase 2: x2 * att -> out (half tiles) ==========
            for b in range(2):
                E = ENG[b]
                attP = att_tiles[b]
                bc4 = bass.AP(attP.tensor, attP.offset, [attP.ap[0], [0, Q // 2], [1, C]])
                for t in range(N_TILES):
                    x2t = x2tiles[(b, t)]
                    for half in range(2):
                        xh = x2t[:, 2048 * half : 2048 * (half + 1)]
                        v = xh.rearrange("p (q c) -> p q c", q=Q // 2)
                        E.tensor_mul(v, v, bc4)
                        TRIG[b].dma_start(out=img_ap(outh, b, t, half), in_=xh)

    nc.compile()
    return nc


def _get_nc():
    if "nc" not in _CACHE:
        _CACHE["nc"] = _build()
    return _CACHE["nc"]


def kernel(**inputs):
    global LAST_RESULTS
    nc = _get_nc()
    ins = {k: np.ascontiguousarray(np.asarray(v, dtype=np.float32)) for k, v in inputs.items()}

    in_maps = []
    for i in range(N_CORES):
        m = {name: ins[name] for name in WEIGHT_SHAPES}
        m["x1"] = np.ascontiguousarray(ins["x1"][B_LOC * i : B_LOC * (i + 1)])
        m["x2"] = np.ascontiguousarray(ins["x2"][B_LOC * i : B_LOC * (i + 1)])
        in_maps.append(m)

    res = run_bass_kernel_spmd(
        nc,
        in_maps,
        core_ids=list(range(N_CORES)),
        trace=bool(int(os.environ.get("BASS_TRACE", "0") or "0")),
    )
    LAST_RESULTS = res
    return np.concatenate([r["out"] for r in res.results], axis=0)


# revision 12
# speedup vs baseline: 1.3444x; 1.3444x over previous
"""Trainium2 Bass kernel for GAP -> tiny Mamba (channel attention) -> broadcast multiply.

Reference computation (per batch):
    pooled = mean(x1 over H,W)                  # [C] ; sequence of length C=512, d_model=1
    att    = mamba(pooled)                      # d_inner=2, d_state=16, dt_rank=1, conv=4
    out    = x2 * att[None, None, :]

Sharding: data-parallel over batch B=16 across 8 cores (2 batches/core), params
replicated. Memory-bound: 48 MiB/core of HBM traffic (x1+x2 reads, out writes)
at ~432 GB/s aggregate (reads and writes share one pool) -> ~117 us DMA floor.

v5 design. v2/v3 traces showed Vector as the co-bottleneck (GAP accumulate+fold
43-54 us + phase-2 35-44 us + chains ~= the DMA floor), which starved the x2
pipeline and serialized the write tail. v4's DMA-accum GAP was a dead end (the
software-DGE accum path moves ~70-100 GB/s and wedges on >8 KiB runs). v5:
  - The ENTIRE GAP (tile accumulate + column fold + 128-partition reduce +
    in_proj/z/conv-tap3 weight application) runs on the TensorEngine as 32
    PSUM-accumulating matmuls per batch: stationary [128,66] f32r (cols 0:2 =
    Xr weights, 32:34 = Z, 64:66 = Ci*cw3, rest zero -- all three projections
    land quadrant-aligned in ONE [66,512] psum bank), moving = each [128,512]
    column block of each x1 tile, bitcast to float32r (4x the f32 matmul
    rate; relaxed precision is ~1e-3 relative, far under the 2e-2 gate).
  - Vector does ONLY the mamba-chain smalls and the phase-2 half-tile
    multiplies (~55 us total, well under the DMA floor).
  - No buffer-reuse stalls feeding reads: x1 streams through a 3-buf pool
    consumed at arrival rate by the PE; x2 has 7 dedicated-ish buffers.
  - Setup stays off Vector (sign of A folded into the negated dt-selector
    half of bsel64); weight DMA descgen split across GpSimd/Scalar queues.
PSUM budget: 2 GAP banks + 4 rotating chain banks + 2 att banks = 8.
Chain scratch [32,4096] per batch replaces the old accumulator aliasing.
"""

import os
import numpy as np

import concourse.bass as bass
import concourse.bacc as bacc
import concourse.tile as tile
from concourse import mybir
from concourse.bass_utils import run_bass_kernel_spmd

F32 = mybir.dt.float32
F32R = mybir.dt.float32r
AF = mybir.ActivationFunctionType
OP = mybir.AluOpType

N_CORES = 8
B_FULL, H, W, C = 16, 64, 64, 512
B_LOC = B_FULL // N_CORES            # 2 batches per core
HW = H * W                           # 4096 spatial positions
Q = 8                                # image rows per partition per stream tile
ROWS_PER_TILE = 128 * Q              # 1024
N_TILES = HW // ROWS_PER_TILE        # 4 tiles per batch image

LN2 = 0.6931471805599453

WEIGHT_SHAPES = {
    "in_proj_w": [4, 1],
    "conv_w": [2, 1, 4],
    "conv_b": [2],
    "x_proj_w": [33, 2],
    "dt_proj_w": [2, 1],
    "dt_proj_b": [2],
    "A_log": [2, 16],
    "Dp": [2],
    "out_proj_w": [1, 2],
}

LAST_RESULTS = None
_CACHE = {}


def _dap(handle, offset, pattern):
    return bass.AP(handle, offset, pattern)


def _build():
    nc = bacc.Bacc(None, target_bir_lowering=False, dynamic_dma_scratch_size=32768)

    x1h = nc.dram_tensor("x1", [B_LOC, H, W, C], F32, kind="ExternalInput")
    x2h = nc.dram_tensor("x2", [B_LOC, H, W, C], F32, kind="ExternalInput")
    wh = {
        name: nc.dram_tensor(name, shape, F32, kind="ExternalInput")
        for name, shape in WEIGHT_SHAPES.items()
    }
    outh = nc.dram_tensor("out", [B_LOC, H, W, C], F32, kind="ExternalOutput")

    # ---- inline 0/1 constants ----
    # [2,64] broadcast selector: cols 0:32 row d -> out rows (d,s) with -1 (for
    # dt; bakes in the sign of A = -exp(A_log)), cols 32:64 with +1 (for g).
    bsel_np = np.zeros((2, 64), np.float32)
    for d in range(2):
        bsel_np[d, 16 * d : 16 * d + 16] = -1.0
        bsel_np[d, 32 + 16 * d : 48 + 16 * d] = 1.0
    bsel_d = nc.inline_tensor(bsel_np, "c_bsel64")
    # [32,2] reduce-s selector: row (d,s) -> col d
    rsel_np = np.zeros((32, 2), np.float32)
    for d in range(2):
        rsel_np[16 * d : 16 * d + 16, d] = 1.0
    rsel_d = nc.inline_tensor(rsel_np, "c_rsel32")

    def img_ap(handle, b, t, half=None):
        off = (b * HW + t * ROWS_PER_TILE) * C
        if half is None:
            return _dap(handle, off, [[Q * C, 128], [1, Q * C]])
        return _dap(handle, off + half * (Q * C // 2), [[Q * C, 128], [1, Q * C // 2]])

    with tile.TileContext(nc) as tc:
        with (
            tc.tile_pool(name="work", bufs=1) as work,
            tc.tile_pool(name="x1pool", bufs=3) as x1pool,
            tc.tile_pool(name="x2pool", bufs=6) as x2pool,
            tc.tile_pool(name="psum_gap", bufs=1, space="PSUM") as psum_gap,
            tc.tile_pool(name="psum", bufs=4, space="PSUM") as psum,
            tc.tile_pool(name="psum_att", bufs=2, space="PSUM") as psum_att,
        ):
            # ========== setup: constants & weight-derived tiles ==========
            # Vector gets NONE of this: its program must start with the data
            # path so nothing upstream gates on weight prep.
            bsel64 = work.tile([2, 64], F32)
            nc.gpsimd.dma_start(out=bsel64[:], in_=bsel_d.ap())
            rsel32 = work.tile([32, 2], F32)
            nc.gpsimd.dma_start(out=rsel32[:], in_=rsel_d.ap())

            # a32 = +exp(A_log) on rows (d,s); sign lives in bsel64 cols 0:32
            a32 = work.tile([32, 1], F32)
            nc.scalar.dma_start(out=a32[:], in_=_dap(wh["A_log"], 0, [[1, 32], [1, 1]]))
            nc.scalar.activation(a32[:], a32[:], AF.Exp)

            cb2 = work.tile([2, 1], F32)       # conv_b
            nc.scalar.dma_start(out=cb2[:], in_=_dap(wh["conv_b"], 0, [[1, 2], [1, 1]]))
            dp2 = work.tile([2, 1], F32)       # Dp
            nc.scalar.dma_start(out=dp2[:], in_=_dap(wh["Dp"], 0, [[1, 2], [1, 1]]))

            # conv taps 0..2 (raw: the xr rows already carry the in_proj weight)
            wq = work.tile([2, 4], F32)
            nc.gpsimd.dma_start(out=wq[:], in_=_dap(wh["conv_w"], 0, [[4, 2], [1, 4]]))

            # stat66w [128,66] f32(bitcast f32r): the one GAP stationary.
            # cols 0:2  = [win0, win1]/HW          -> psum rows 0:2  (Xr)
            # cols 32:34= [wz0, wz1]/HW            -> psum rows 32:34 (Z)
            # cols 64:66= [win0*cw03, win1*cw13]/HW-> psum rows 64:66 (Ci)
            # everything else zero; all three outputs are quadrant-aligned.
            w6 = work.tile([1, 6], F32)
            nc.gpsimd.dma_start(out=w6[0:1, 0:4], in_=_dap(wh["in_proj_w"], 0, [[0, 1], [1, 4]]))
            cw3 = work.tile([1, 2], F32)
            nc.gpsimd.dma_start(out=cw3[:], in_=_dap(wh["conv_w"], 3, [[0, 1], [4, 2]]))
            nc.gpsimd.tensor_mul(w6[0:1, 4:6], w6[0:1, 0:2], cw3[:])
            nc.scalar.mul(w6[:], w6[:], 1.0 / HW)
            w66 = work.tile([1, 66], F32)
            nc.gpsimd.memset(w66[:], 0.0)
            nc.gpsimd.tensor_copy(w66[0:1, 0:2], w6[0:1, 0:2])
            nc.gpsimd.tensor_copy(w66[0:1, 32:34], w6[0:1, 2:4])
            nc.gpsimd.tensor_copy(w66[0:1, 64:66], w6[0:1, 4:6])
            stat66w = work.tile([128, 66], F32R)
            nc.gpsimd.partition_broadcast(stat66w[:], w66[:].bitcast(F32R))

            # stat66 [3,66]: moving rows (xconv d0, xconv d1, ones).
            # cols 0-1:  dt_pre rows (d): xp_dt[d']*dtw[d] (+ dtb[d] via ones row)
            # cols 2-33:  B rows (d,s): xp_B[s, d']
            # cols 34-65: C rows (d,s): xp_C[s, d']
            stat66 = work.tile([3, 66], F32)
            nc.gpsimd.memset(stat66[:], 0.0)
            xpdt2 = work.tile([2, 1], F32)
            nc.scalar.dma_start(out=xpdt2[:], in_=_dap(wh["x_proj_w"], 0, [[1, 2], [1, 1]]))
            dtwbc = work.tile([2, 2], F32)
            nc.scalar.dma_start(out=dtwbc[:], in_=_dap(wh["dt_proj_w"], 0, [[0, 2], [1, 2]]))
            nc.scalar.mul(stat66[0:2, 0:2], dtwbc[:], xpdt2[:])
            nc.scalar.dma_start(out=stat66[2:3, 0:2], in_=_dap(wh["dt_proj_b"], 0, [[0, 1], [1, 2]]))
            for d in range(2):
                nc.scalar.dma_start(
                    out=stat66[0:2, 2 + 16 * d : 18 + 16 * d],
                    in_=_dap(wh["x_proj_w"], 2, [[1, 2], [2, 16]]),
                )
                nc.scalar.dma_start(
                    out=stat66[0:2, 34 + 16 * d : 50 + 16 * d],
                    in_=_dap(wh["x_proj_w"], 34, [[1, 2], [2, 16]]),
                )

            # wout_bc [2,128]: every col = out_proj_w; fuses out_proj with the
            # broadcast of att to 128 partitions.
            wout2 = work.tile([2, 1], F32)
            nc.scalar.dma_start(out=wout2[:], in_=_dap(wh["out_proj_w"], 0, [[1, 2], [1, 1]]))
            wout_bc = work.tile([2, 128], F32)
            nc.gpsimd.tensor_copy(
                wout_bc[:], bass.AP(wout2.tensor, wout2.offset, [wout2.ap[0], [0, 128]])
            )

            # xconv moving tiles [3, C]: rows 0-1 = silu(conv), row 2 = ones.
            xconv3 = []
            for b in range(2):
                xc = work.tile([3, C], F32, tag=f"xconv{b}")
                nc.gpsimd.memset(xc[:], 1.0)     # row 2 stays 1.0
                xconv3.append(xc)

            # chain scratch: [32, 8*512] column slots per batch
            scr = []
            for b in range(2):
                scr_b = work.tile([32, 8 * C], F32, tag=f"scr{b}")
                scr.append(scr_b)

            ENG = [nc.vector, nc.vector]
            TRIG = [nc.scalar, nc.scalar]

            # ========== phase 1: reads (x1 priority, then x2) ==========
            x1tiles = {}
            for b in range(2):
                for t in range(N_TILES):
                    xt = x1pool.tile([128, Q * C], F32R, tag="x1t")
                    nc.sync.dma_start(out=xt[:], in_=img_ap(x1h, b, t).bitcast(F32R))
                    x1tiles[(b, t)] = xt
            x2tiles = {}
            for b in range(2):
                for t in range(N_TILES):
                    x2t = x2pool.tile([128, Q * C], F32, tag="x2t")
                    nc.sync.dma_start(out=x2t[:], in_=img_ap(x2h, b, t))
                    x2tiles[(b, t)] = x2t

            # GAP entirely on the PE: 32 accumulating f32r matmuls per batch
            # into one [66,512] psum bank.
            gapP = []
            for b in range(2):
                gp = psum_gap.tile([66, C], F32, tag=f"gap{b}")
                for t in range(N_TILES):
                    xt = x1tiles[(b, t)]
                    for k in range(8):
                        nc.tensor.matmul(
                            gp[:],
                            stat66w[:],
                            xt[:, C * k : C * (k + 1)],
                            start=(t == 0 and k == 0),
                            stop=(t == N_TILES - 1 and k == 7),
                        )
                gapP.append(gp)

            # ========== per-batch mamba chain ==========
            def slot(b, k, p=32):
                return scr[b][0:p, 512 * k : 512 * (k + 1)]

            def mamba(b):
                E = ENG[b]
                xc = xconv3[b]
                gp = gapP[b]
                gapXr = gp[0:2, :]
                gapZ = gp[32:34, :]
                gapCi = gp[64:66, :]
                # causal conv: cacc = cinit; taps 2,1,0 read xr straight from PSUM
                cacc = slot(b, 5, 2)
                E.tensor_copy(cacc, gapCi)
                for j in (2, 1, 0):
                    s = 3 - j
                    E.scalar_tensor_tensor(
                        cacc[:, s:C], gapXr[:, 0 : C - s], wq[:, j : j + 1],
                        cacc[:, s:C], op0=OP.mult, op1=OP.add,
                    )
                # xconv = silu(conv + conv_b); sz = silu(z) straight from PSUM
                sz = slot(b, 6, 2)
                nc.scalar.activation(xc[0:2, :], cacc, AF.Silu, bias=cb2[:])
                nc.scalar.activation(sz, gapZ, AF.Silu)
                # x_proj + dt_proj(+bias): three base-0 psum tiles
                xdtP = psum.tile([2, C], F32, tag="pp")
                nc.tensor.matmul(xdtP[:], stat66[:, 0:2], xc[:], start=True, stop=True)
                xbP = psum.tile([32, C], F32, tag="pp")
                nc.tensor.matmul(xbP[:], stat66[:, 2:34], xc[:], start=True, stop=True)
                xcP = psum.tile([32, C], F32, tag="pp")
                nc.tensor.matmul(xcP[:], stat66[:, 34:66], xc[:], start=True, stop=True)
                bm = slot(b, 0)
                E.tensor_copy(bm, xbP[:])
                # dt = softplus(dt_pre) ~= ln2 + x/2 + x^2*(1/8 - x^2/192)
                t2a = slot(b, 3, 2)
                t2b = slot(b, 4, 2)
                t2c = slot(b, 5, 2)     # cacc is dead after the silu
                dt2 = slot(b, 7, 2)
                E.tensor_copy(t2a, xdtP[:])
                E.tensor_mul(t2b, t2a, t2a)
                E.tensor_scalar(t2c, t2b, -1.0 / 192.0, 0.125, op0=OP.mult, op1=OP.add)
                E.tensor_mul(t2c, t2c, t2b)
                E.tensor_scalar(t2a, t2a, 0.5, LN2, op0=OP.mult, op1=OP.add)
                E.tensor_add(dt2, t2c, t2a)
                g2 = slot(b, 5, 2)      # t2c is dead after dt2
                E.tensor_mul(g2, dt2, xc[0:2, :])        # g = dt*xconv
                # broadcast dt,g to (d,s) lanes; the dt selector is negated so
                # da = exp(a32 * (-dt)) = exp(A * dt) with a32 = +exp(A_log)
                dag1P = psum.tile([32, C], F32, tag="pp")
                nc.tensor.matmul(dag1P[:], bsel64[:, 0:32], dt2, start=True, stop=True)
                dag2P = psum.tile([32, C], F32, tag="pp")
                nc.tensor.matmul(dag2P[:], bsel64[:, 32:64], g2, start=True, stop=True)
                da = slot(b, 7)         # dt2 rows are dead after dag1P
                nc.scalar.activation(da, dag1P[:], AF.Exp, scale=a32[:])
                dbu = slot(b, 1)
                E.tensor_mul(dbu, dag2P[:], bm)
                # selective scan h[:,t] = dA[:,t]*h[:,t-1] + dBu[:,t]
                h = slot(b, 2)
                E.tensor_tensor_scan(h, da, dbu, 0.0, op0=OP.mult, op1=OP.add)
                hc = slot(b, 1)         # dbu dead after the scan
                E.tensor_mul(hc, h, xcP[:])
                y2P = psum.tile([2, C], F32, tag="pp")
                nc.tensor.matmul(y2P[:], rsel32[:], hc, start=True, stop=True)
                # y = (y + Dp*xconv) * silu(z); att = out_proj(y) broadcast
                yg = slot(b, 3, 2)      # t2a dead after dt2
                E.scalar_tensor_tensor(yg, xc[0:2, :], dp2[:], y2P[:], op0=OP.mult, op1=OP.add)
                E.tensor_mul(yg, yg, sz)
                # att lives in its own 2-bank pool: it stays live through all
                # of the batch's phase-2 multiplies.
                attP = psum_att.tile([128, C], F32, tag="att")
                nc.tensor.matmul(attP[:], wout_bc[:], yg, start=True, stop=True)
                return attP

            att_tiles = [mamba(0), mamba(1)]

            # ========== phase 2: x2 * att -> out (half tiles) ==========
            for b in range(2):
                E = ENG[b]
                attP = att_tiles[b]
                bc4 = bass.AP(attP.tensor, attP.offset, [attP.ap[0], [0, Q // 2], [1, C]])
                for t in range(N_TILES):
                    x2t = x2tiles[(b, t)]
                    for half in range(2):
                        xh = x2t[:, 2048 * half : 2048 * (half + 1)]
                        v = xh.rearrange("p (q c) -> p q c", q=Q // 2)
                        E.tensor_mul(v, v, bc4)
                        TRIG[b].dma_start(out=img_ap(outh, b, t, half), in_=xh)

    nc.compile()
    return nc


def _get_nc():
    if "nc" not in _CACHE:
        _CACHE["nc"] = _build()
    return _CACHE["nc"]


def kernel(**inputs):
    global LAST_RESULTS
    nc = _get_nc()
    ins = {k: np.ascontiguousarray(np.asarray(v, dtype=np.float32)) for k, v in inputs.items()}

    in_maps = []
    for i in range(N_CORES):
        m = {name: ins[name] for name in WEIGHT_SHAPES}
        m["x1"] = np.ascontiguousarray(ins["x1"][B_LOC * i : B_LOC * (i + 1)])
        m["x2"] = np.ascontiguousarray(ins["x2"][B_LOC * i : B_LOC * (i + 1)])
        in_maps.append(m)

    res = run_bass_kernel_spmd(
        nc,
        in_maps,
        core_ids=list(range(N_CORES)),
        trace=bool(int(os.environ.get("BASS_TRACE", "0") or "0")),
    )
    LAST_RESULTS = res
    return np.concatenate([r["out"] for r in res.results], axis=0)


# revision 18
# speedup vs baseline: 1.3583x; 1.0103x over previous
"""Trainium2 Bass kernel for GAP -> tiny Mamba (channel attention) -> broadcast multiply.

Reference computation (per batch):
    pooled = mean(x1 over H,W)                  # [C] ; sequence of length C=512, d_model=1
    att    = mamba(pooled)                      # d_inner=2, d_state=16, dt_rank=1, conv=4
    out    = x2 * att[None, None, :]

Sharding: data-parallel over batch B=16 across 8 cores (2 batches/core), params
replicated. Memory-bound: 48 MiB/core of HBM traffic (x1+x2 reads, out writes)
at ~432 GB/s aggregate (reads and writes share one pool) -> ~117 us DMA floor.

v5 design. v2/v3 traces showed Vector as the co-bottleneck (GAP accumulate+fold
43-54 us + phase-2 35-44 us + chains ~= the DMA floor), which starved the x2
pipeline and serialized the write tail. v4's DMA-accum GAP was a dead end (the
software-DGE accum path moves ~70-100 GB/s and wedges on >8 KiB runs). v5:
  - The ENTIRE GAP (tile accumulate + column fold + 128-partition reduce +
    in_proj/z/conv-tap3 weight application) runs on the TensorEngine as 32
    PSUM-accumulating matmuls per batch: stationary [128,66] f32r (cols 0:2 =
    Xr weights, 32:34 = Z, 64:66 = Ci*cw3, rest zero -- all three projections
    land quadrant-aligned in ONE [66,512] psum bank), moving = each [128,512]
    column block of each x1 tile, bitcast to float32r (4x the f32 matmul
    rate; relaxed precision is ~1e-3 relative, far under the 2e-2 gate).
  - Vector does ONLY the mamba-chain smalls and the phase-2 half-tile
    multiplies (~55 us total, well under the DMA floor).
  - No buffer-reuse stalls feeding reads: x1 streams through a 3-buf pool
    consumed at arrival rate by the PE; x2 has 7 dedicated-ish buffers.
  - Setup stays off Vector (sign of A folded into the negated dt-selector
    half of bsel64); weight DMA descgen split across GpSimd/Scalar queues.
PSUM budget: 2 GAP banks + 4 rotating chain banks + 2 att banks = 8.
Chain scratch [32,4096] per batch replaces the old accumulator aliasing.
"""

import os
import numpy as np

import concourse.bass as bass
import concourse.bacc as bacc
import concourse.tile as tile
from concourse import mybir
from concourse.bass_utils import run_bass_kernel_spmd

F32 = mybir.dt.float32
F32R = mybir.dt.float32r
AF = mybir.ActivationFunctionType
OP = mybir.AluOpType

N_CORES = 8
B_FULL, H, W, C = 16, 64, 64, 512
B_LOC = B_FULL // N_CORES            # 2 batches per core
HW = H * W                           # 4096 spatial positions
Q = 8                                # image rows per partition per stream tile
ROWS_PER_TILE = 128 * Q              # 1024
N_TILES = HW // ROWS_PER_TILE        # 4 tiles per batch image

LN2 = 0.6931471805599453

WEIGHT_SHAPES = {
    "in_proj_w": [4, 1],
    "conv_w": [2, 1, 4],
    "conv_b": [2],
    "x_proj_w": [33, 2],
    "dt_proj_w": [2, 1],
    "dt_proj_b": [2],
    "A_log": [2, 16],
    "Dp": [2],
    "out_proj_w": [1, 2],
}

LAST_RESULTS = None
_CACHE = {}


def _dap(handle, offset, pattern):
    return bass.AP(handle, offset, pattern)


def _build():
    nc = bacc.Bacc(None, target_bir_lowering=False, dynamic_dma_scratch_size=32768)

    x1h = nc.dram_tensor("x1", [B_LOC, H, W, C], F32, kind="ExternalInput")
    x2h = nc.dram_tensor("x2", [B_LOC, H, W, C], F32, kind="ExternalInput")
    wh = {
        name: nc.dram_tensor(name, shape, F32, kind="ExternalInput")
        for name, shape in WEIGHT_SHAPES.items()
    }
    outh = nc.dram_tensor("out", [B_LOC, H, W, C], F32, kind="ExternalOutput")

    # ---- inline 0/1 constants ----
    # [2,64] broadcast selector: cols 0:32 row d -> out rows (d,s) with -1 (for
    # dt; bakes in the sign of A = -exp(A_log)), cols 32:64 with +1 (for g).
    bsel_np = np.zeros((2, 64), np.float32)
    for d in range(2):
        bsel_np[d, 16 * d : 16 * d + 16] = -1.0
        bsel_np[d, 32 + 16 * d : 48 + 16 * d] = 1.0
    bsel_d = nc.inline_tensor(bsel_np, "c_bsel64")
    # [32,2] reduce-s selector: row (d,s) -> col d
    rsel_np = np.zeros((32, 2), np.float32)
    for d in range(2):
        rsel_np[16 * d : 16 * d + 16, d] = 1.0
    rsel_d = nc.inline_tensor(rsel_np, "c_rsel32")

    def img_ap(handle, b, t, half=None):
        off = (b * HW + t * ROWS_PER_TILE) * C
        if half is None:
            return _dap(handle, off, [[Q * C, 128], [1, Q * C]])
        return _dap(handle, off + half * (Q * C // 2), [[Q * C, 128], [1, Q * C // 2]])

    with tile.TileContext(nc) as tc:
        with (
            tc.tile_pool(name="work", bufs=1) as work,
            tc.tile_pool(name="x1pool", bufs=3) as x1pool,
            tc.tile_pool(name="x2pool", bufs=6) as x2pool,
            tc.tile_pool(name="psum_gap", bufs=1, space="PSUM") as psum_gap,
            tc.tile_pool(name="psum", bufs=4, space="PSUM") as psum,
            tc.tile_pool(name="psum_att", bufs=2, space="PSUM") as psum_att,
        ):
            # ========== setup: constants & weight-derived tiles ==========
            # Vector gets NONE of this: its program must start with the data
            # path so nothing upstream gates on weight prep.
            bsel64 = work.tile([2, 64], F32)
            nc.gpsimd.dma_start(out=bsel64[:], in_=bsel_d.ap())
            rsel32 = work.tile([32, 2], F32)
            nc.gpsimd.dma_start(out=rsel32[:], in_=rsel_d.ap())

            # a32 = +exp(A_log) on rows (d,s); sign lives in bsel64 cols 0:32
            a32 = work.tile([32, 1], F32)
            nc.scalar.dma_start(out=a32[:], in_=_dap(wh["A_log"], 0, [[1, 32], [1, 1]]))
            nc.scalar.activation(a32[:], a32[:], AF.Exp)

            cb2 = work.tile([2, 1], F32)       # conv_b
            nc.scalar.dma_start(out=cb2[:], in_=_dap(wh["conv_b"], 0, [[1, 2], [1, 1]]))
            dp2 = work.tile([2, 1], F32)       # Dp
            nc.scalar.dma_start(out=dp2[:], in_=_dap(wh["Dp"], 0, [[1, 2], [1, 1]]))

            # conv taps 0..2 (raw: the xr rows already carry the in_proj weight)
            wq = work.tile([2, 4], F32)
            nc.gpsimd.dma_start(out=wq[:], in_=_dap(wh["conv_w"], 0, [[4, 2], [1, 4]]))

            # stat66w [128,66] f32r: the one GAP stationary.
            # cols 0:2  = [win0, win1]/HW          -> psum rows 0:2  (Xr)
            # cols 32:34= [wz0, wz1]/HW            -> psum rows 32:34 (Z)
            # cols 64:66= [win0*cw03, win1*cw13]/HW-> psum rows 64:66 (Ci)
            # everything else zero; all three outputs are quadrant-aligned.
            # Built on Vector (free until the chains start; GpSimd's tensor-op
            # library reload costs ~15-20 us) and broadcast to 128 partitions
            # with a 0-stride SBUF->SBUF DMA on the scalar ring.
            w66 = work.tile([1, 66], F32)
            nc.vector.memset(w66[:], 0.0)
            nc.scalar.dma_start(out=w66[0:1, 0:2], in_=_dap(wh["in_proj_w"], 0, [[0, 1], [1, 2]]))
            nc.scalar.dma_start(out=w66[0:1, 32:34], in_=_dap(wh["in_proj_w"], 2, [[0, 1], [1, 2]]))
            nc.scalar.dma_start(out=w66[0:1, 64:66], in_=_dap(wh["in_proj_w"], 0, [[0, 1], [1, 2]]))
            cw3 = work.tile([1, 2], F32)
            nc.scalar.dma_start(out=cw3[:], in_=_dap(wh["conv_w"], 3, [[0, 1], [4, 2]]))
            nc.vector.tensor_mul(w66[0:1, 64:66], w66[0:1, 64:66], cw3[:])
            nc.vector.tensor_scalar_mul(w66[:], w66[:], 1.0 / HW)
            stat66w = work.tile([128, 66], F32R)
            nc.scalar.dma_start(
                out=stat66w[:],
                in_=bass.AP(w66.tensor, w66.offset, [[1, 1], [0, 128], [1, 66]]).bitcast(F32R),
            )

            # stat66 [3,66]: moving rows (xconv d0, xconv d1, ones).
            # cols 0-1:  dt_pre rows (d): xp_dt[d']*dtw[d] (+ dtb[d] via ones row)
            # cols 2-33:  B rows (d,s): xp_B[s, d']
            # cols 34-65: C rows (d,s): xp_C[s, d']
            stat66 = work.tile([3, 66], F32)
            nc.gpsimd.memset(stat66[:], 0.0)
            xpdt2 = work.tile([2, 1], F32)
            nc.scalar.dma_start(out=xpdt2[:], in_=_dap(wh["x_proj_w"], 0, [[1, 2], [1, 1]]))
            dtwbc = work.tile([2, 2], F32)
            nc.scalar.dma_start(out=dtwbc[:], in_=_dap(wh["dt_proj_w"], 0, [[0, 2], [1, 2]]))
            nc.scalar.mul(stat66[0:2, 0:2], dtwbc[:], xpdt2[:])
            nc.scalar.dma_start(out=stat66[2:3, 0:2], in_=_dap(wh["dt_proj_b"], 0, [[0, 1], [1, 2]]))
            for d in range(2):
                nc.scalar.dma_start(
                    out=stat66[0:2, 2 + 16 * d : 18 + 16 * d],
                    in_=_dap(wh["x_proj_w"], 2, [[1, 2], [2, 16]]),
                )
                nc.scalar.dma_start(
                    out=stat66[0:2, 34 + 16 * d : 50 + 16 * d],
                    in_=_dap(wh["x_proj_w"], 34, [[1, 2], [2, 16]]),
                )

            # wout_bc [2,128]: every col = out_proj_w; fuses out_proj with the
            # broadcast of att to 128 partitions.
            wout2 = work.tile([2, 1], F32)
            nc.scalar.dma_start(out=wout2[:], in_=_dap(wh["out_proj_w"], 0, [[1, 2], [1, 1]]))
            wout_bc = work.tile([2, 128], F32)
            nc.vector.tensor_copy(
                wout_bc[:], bass.AP(wout2.tensor, wout2.offset, [wout2.ap[0], [0, 128]])
            )

            # xconv moving tiles [3, C]: rows 0-1 = silu(conv), row 2 = ones.
            xconv3 = []
            for b in range(2):
                xc = work.tile([3, C], F32, tag=f"xconv{b}")
                nc.gpsimd.memset(xc[:], 1.0)     # row 2 stays 1.0
                xconv3.append(xc)

            # chain scratch: [32, 8*512] column slots per batch
            scr = []
            for b in range(2):
                scr_b = work.tile([32, 8 * C], F32, tag=f"scr{b}")
                scr.append(scr_b)

            ENG = [nc.vector, nc.vector]
            TRIG = [nc.scalar, nc.scalar]

            # ========== phase 1: reads (x1 priority, then x2) ==========
            x1tiles = {}
            for b in range(2):
                for t in range(N_TILES):
                    xt = x1pool.tile([128, Q * C], F32R, tag="x1t")
                    nc.sync.dma_start(out=xt[:], in_=img_ap(x1h, b, t).bitcast(F32R))
                    x1tiles[(b, t)] = xt
            x2tiles = {}
            for b in range(2):
                for t in range(N_TILES):
                    x2t = x2pool.tile([128, Q * C], F32, tag="x2t")
                    nc.sync.dma_start(out=x2t[:], in_=img_ap(x2h, b, t))
                    x2tiles[(b, t)] = x2t

            # GAP entirely on the PE: 32 accumulating f32r matmuls per batch
            # into one [66,512] psum bank.
            gapP = []
            for b in range(2):
                gp = psum_gap.tile([66, C], F32, tag=f"gap{b}")
                for t in range(N_TILES):
                    xt = x1tiles[(b, t)]
                    for k in range(8):
                        nc.tensor.matmul(
                            gp[:],
                            stat66w[:],
                            xt[:, C * k : C * (k + 1)],
                            start=(t == 0 and k == 0),
                            stop=(t == N_TILES - 1 and k == 7),
                        )
                gapP.append(gp)

            # ========== per-batch mamba chain ==========
            def slot(b, k, p=32):
                return scr[b][0:p, 512 * k : 512 * (k + 1)]

            def mamba(b):
                E = ENG[b]
                xc = xconv3[b]
                gp = gapP[b]
                gapXr = gp[0:2, :]
                gapZ = gp[32:34, :]
                gapCi = gp[64:66, :]
                # sz = silu(z) = z/(1+exp(-z)): Exp on scalar + reciprocal on
                # vector. Only Exp ever touches the activation table -> the
                # per-switch ACT_TABLE_LOADs (1.3 us each) disappear.
                ez = slot(b, 4, 2)
                sgz = slot(b, 3, 2)
                sz = slot(b, 6, 2)
                nc.scalar.activation(ez, gapZ, AF.Exp, scale=-1.0)
                E.tensor_scalar_add(ez, ez, 1.0)
                E.reciprocal(sgz, ez)
                E.tensor_mul(sz, sgz, gapZ)
                # causal conv: cacc = cinit; taps 2,1,0 read xr straight from PSUM
                cacc = slot(b, 5, 2)
                E.tensor_copy(cacc, gapCi)
                for j in (2, 1, 0):
                    s = 3 - j
                    E.scalar_tensor_tensor(
                        cacc[:, s:C], gapXr[:, 0 : C - s], wq[:, j : j + 1],
                        cacc[:, s:C], op0=OP.mult, op1=OP.add,
                    )
                # xconv = silu(cacc + conv_b), same exp+reciprocal route
                xcb = slot(b, 0, 2)     # bm's slot, dead until after x_proj
                E.tensor_scalar_add(xcb, cacc, cb2[:])
                ec = slot(b, 4, 2)      # ez dead
                nc.scalar.activation(ec, xcb, AF.Exp, scale=-1.0)
                E.tensor_scalar_add(ec, ec, 1.0)
                sgc = slot(b, 3, 2)     # sgz dead
                E.reciprocal(sgc, ec)
                E.tensor_mul(xc[0:2, :], xcb, sgc)
                # x_proj + dt_proj(+bias): three base-0 psum tiles
                xdtP = psum.tile([2, C], F32, tag="pp")
                nc.tensor.matmul(xdtP[:], stat66[:, 0:2], xc[:], start=True, stop=True)
                xbP = psum.tile([32, C], F32, tag="pp")
                nc.tensor.matmul(xbP[:], stat66[:, 2:34], xc[:], start=True, stop=True)
                xcP = psum.tile([32, C], F32, tag="pp")
                nc.tensor.matmul(xcP[:], stat66[:, 34:66], xc[:], start=True, stop=True)
                bm = slot(b, 0)
                E.tensor_copy(bm, xbP[:])
                # dt = softplus(dt_pre) ~= ln2 + x/2 + x^2*(1/8 - x^2/192)
                t2a = slot(b, 3, 2)
                t2b = slot(b, 4, 2)
                t2c = slot(b, 5, 2)     # cacc is dead after the silu
                dt2 = slot(b, 7, 2)
                E.tensor_copy(t2a, xdtP[:])
                E.tensor_mul(t2b, t2a, t2a)
                E.tensor_scalar(t2c, t2b, -1.0 / 192.0, 0.125, op0=OP.mult, op1=OP.add)
                E.tensor_mul(t2c, t2c, t2b)
                E.tensor_scalar(t2a, t2a, 0.5, LN2, op0=OP.mult, op1=OP.add)
                E.tensor_add(dt2, t2c, t2a)
                g2 = slot(b, 5, 2)      # t2c is dead after dt2
                E.tensor_mul(g2, dt2, xc[0:2, :])        # g = dt*xconv
                # broadcast dt,g to (d,s) lanes; the dt selector is negated so
                # da = exp(a32 * (-dt)) = exp(A * dt) with a32 = +exp(A_log)
                dag1P = psum.tile([32, C], F32, tag="pp")
                nc.tensor.matmul(dag1P[:], bsel64[:, 0:32], dt2, start=True, stop=True)
                dag2P = psum.tile([32, C], F32, tag="pp")
                nc.tensor.matmul(dag2P[:], bsel64[:, 32:64], g2, start=True, stop=True)
                da = slot(b, 7)         # dt2 rows are dead after dag1P
                nc.scalar.activation(da, dag1P[:], AF.Exp, scale=a32[:])
                dbu = slot(b, 1)
                E.tensor_mul(dbu, dag2P[:], bm)
                # selective scan h[:,t] = dA[:,t]*h[:,t-1] + dBu[:,t]
                h = slot(b, 2)
                E.tensor_tensor_scan(h, da, dbu, 0.0, op0=OP.mult, op1=OP.add)
                hc = slot(b, 1)         # dbu dead after the scan
                E.tensor_mul(hc, h, xcP[:])
                y2P = psum.tile([2, C], F32, tag="pp")
                nc.tensor.matmul(y2P[:], rsel32[:], hc, start=True, stop=True)
                # y = (y + Dp*xconv) * silu(z); att = out_proj(y) broadcast
                yg = slot(b, 3, 2)      # t2a dead after dt2
                E.scalar_tensor_tensor(yg, xc[0:2, :], dp2[:], y2P[:], op0=OP.mult, op1=OP.add)
                E.tensor_mul(yg, yg, sz)
                # att lives in its own 2-bank pool: it stays live through all
                # of the batch's phase-2 multiplies.
                attP = psum_att.tile([128, C], F32, tag="att")
                nc.tensor.matmul(attP[:], wout_bc[:], yg, start=True, stop=True)
                return attP

            att_tiles = [mamba(0), mamba(1)]

            # ========== phase 2: x2 * att -> out (half tiles) ==========
            for b in range(2):
                E = ENG[b]
                attP = att_tiles[b]
                bc4 = bass.AP(attP.tensor, attP.offset, [attP.ap[0], [0, Q // 2], [1, C]])
                for t in range(N_TILES):
                    x2t = x2tiles[(b, t)]
                    for half in range(2):
                        xh = x2t[:, 2048 * half : 2048 * (half + 1)]
                        v = xh.rearrange("p (q c) -> p q c", q=Q // 2)
                        E.tensor_mul(v, v, bc4)
                        TRIG[b].dma_start(out=img_ap(outh, b, t, half), in_=xh)

    nc.compile()
    return nc


def _get_nc():
    if "nc" not in _CACHE:
        _CACHE["nc"] = _build()
    return _CACHE["nc"]


def kernel(**inputs):
    global LAST_RESULTS
    nc = _get_nc()
    ins = {k: np.ascontiguousarray(np.asarray(v, dtype=np.float32)) for k, v in inputs.items()}

    in_maps = []
    for i in range(N_CORES):
        m = {name: ins[name] for name in WEIGHT_SHAPES}
        m["x1"] = np.ascontiguousarray(ins["x1"][B_LOC * i : B_LOC * (i + 1)])
        m["x2"] = np.ascontiguousarray(ins["x2"][B_LOC * i : B_LOC * (i + 1)])
        in_maps.append(m)

    res = run_bass_kernel_spmd(
        nc,
        in_maps,
        core_ids=list(range(N_CORES)),
        trace=bool(int(os.environ.get("BASS_TRACE", "0") or "0")),
    )
    LAST_RESULTS = res
    return np.concatenate([r["out"] for r in res.results], axis=0)
